# revision 2
# baseline (speedup 1.0000x reference)
"""nn_Matcher Trainium2 kernel — fp16 device path, SWDGE DMA flood.

Measured model for this axon/trn2 environment (see memory notes):
- each dma_start is ~one ~30GB/s stream; gpsimd (SWDGE) queue runs ~6
  streams concurrently, HWDGE rings are serial; per-descriptor ~0.2us.
- per-exec dispatch overhead ~300us dwarfs device time; true device
  time measured via reps-unrolled builds at ~70-90us/exec.

Structure per core (GPC=4 graphs, 3 tensors, m1=82, m2=83, K=64):
- ch0 (out[t,g,0][i,jK+k] = p1[t,g,i,k]): load p1 rows [m1, K] to SBUF,
  one-shot free-dim stride-0 broadcast copy (DVE/ACT alternating) into
  e0 [m1, m2*K], one whole-tile gpsimd DMA out.
- ch1 (out[t,g,1][i,jK+k] = p2[t,g,j,k]): single HBM->HBM row-broadcast
  DMA per tile (DRAM [1, m2K] -> [m1, m2K], stride-0 partition loop is
  legal for DRAM sources), engine-free.
- fp16 on device (tolerance 2e-2 >> fp16 rounding 2^-11); host pads
  inputs to fp16 and upcasts outputs to fp32.
"""
import sys

if '/opt/trn_rl_repo' not in sys.path:
    sys.path.insert(0, '/opt/trn_rl_repo')

import numpy as np

G, N1, N2, K = 32, 2048, 2048, 64
N_CORES = 8
GPC = G // N_CORES

_cache = {}


def _build(m1, m2):
    from concourse import bacc
    import concourse.tile as tile
    import concourse.mybir as mybir

    F = mybir.dt.float16
    m2k = m2 * K
    nc = bacc.Bacc("TRN2", target_bir_lowering=False, debug=False)
    p1 = nc.dram_tensor("p1", [3, GPC, m1, K], F, kind="ExternalInput")
    p2f = nc.dram_tensor("p2f", [3, GPC, 1, m2k], F, kind="ExternalInput")
    out = nc.dram_tensor("out", [3, GPC, 2, m1, m2k], F,
                         kind="ExternalOutput")

    with tile.TileContext(nc) as tc:
        with tc.tile_pool(name="in1", bufs=2) as pin1, \
             tc.tile_pool(name="exp0", bufs=3) as pexp0:
            it = 0
            for t in range(3):
                for g in range(GPC):
                    # ch1: one engine-free HBM->HBM row-broadcast DMA
                    nc.gpsimd.dma_start(
                        out[t, g, 1],
                        p2f[t, g].broadcast_to([m1, m2k]))

                    # ch0: load + one-shot broadcast copy + one DMA
                    t1 = pin1.tile([m1, K], F)
                    nc.sync.dma_start(t1[:], p1[t, g])
                    e0 = pexp0.tile([m1, m2k], F)
                    nc.vector.tensor_copy(
                        e0[:], t1[:].unsqueeze(1).broadcast_to([m1, m2, K]))
                    nc.gpsimd.dma_start(out[t, g, 0], e0[:])
                    it += 1
    nc.compile()
    return nc


def _pad_groups_np(x, ids, m):
    counts = np.bincount(ids, minlength=G)
    starts = np.cumsum(counts) - counts
    pos = np.arange(ids.shape[0]) - starts[ids]
    outp = np.zeros((G, m, x.shape[1]), np.float16)
    outp[ids, pos] = x.astype(np.float16)
    return outp


def _make_in_maps(inputs):
    ids1 = np.asarray(inputs['ids1']).astype(np.int64)
    ids2 = np.asarray(inputs['ids2']).astype(np.int64)
    m1 = int(inputs['maxcount1'])
    m2 = int(inputs['maxcount2'])
    xs1 = [np.asarray(inputs[n]) for n in ('x_f_1', 'x_e_1', 'x_v_1')]
    xs2 = [np.asarray(inputs[n]) for n in ('x_f_2', 'x_e_2', 'x_v_2')]
    pad1 = np.stack([_pad_groups_np(x, ids1, m1) for x in xs1])
    pad2f = np.stack([_pad_groups_np(x, ids2, m2) for x in xs2]).reshape(
        3, G, 1, m2 * K)
    in_maps = [
        {"p1": np.ascontiguousarray(pad1[:, c * GPC:(c + 1) * GPC]),
         "p2f": np.ascontiguousarray(pad2f[:, c * GPC:(c + 1) * GPC])}
        for c in range(N_CORES)
    ]
    return in_maps, m1, m2


def kernel(**inputs):
    from concourse.bass_utils import run_bass_kernel_spmd

    in_maps, m1, m2 = _make_in_maps(inputs)
    key = (m1, m2)
    if key not in _cache:
        _cache[key] = _build(m1, m2)
    nc = _cache[key]

    res = run_bass_kernel_spmd(nc, in_maps, core_ids=list(range(N_CORES)))

    full = np.empty((3, G, 2, m1, m2, K), np.float32)
    for c in range(N_CORES):
        full[:, c * GPC:(c + 1) * GPC] = np.asarray(
            res.results[c]["out"]).astype(np.float32).reshape(
            3, GPC, 2, m1, m2, K)
    return full[0], full[1], full[2]


# revision 3
# speedup vs baseline: 1.6072x; 1.6072x over previous
"""nn_Matcher Trainium2 kernel — fp16 device path, SWDGE DMA flood.

Measured model for this axon/trn2 environment (see memory notes):
- each dma_start is ~one ~30GB/s stream; gpsimd (SWDGE) queue runs ~6
  streams concurrently, HWDGE rings are serial; per-descriptor ~0.2us.
- per-exec dispatch overhead ~300us dwarfs device time; true device
  time measured via reps-unrolled builds at ~70-90us/exec.

Structure per core (GPC=4 graphs, 3 tensors, m1=82, m2=83, K=64):
- ch0 (out[t,g,0][i,jK+k] = p1[t,g,i,k]): load p1 rows [m1, K] to SBUF,
  one-shot free-dim stride-0 broadcast copy (DVE/ACT alternating) into
  e0 [m1, m2*K], one whole-tile gpsimd DMA out.
- ch1 (out[t,g,1][i,jK+k] = p2[t,g,j,k]): single HBM->HBM row-broadcast
  DMA per tile (DRAM [1, m2K] -> [m1, m2K], stride-0 partition loop is
  legal for DRAM sources), engine-free.
- fp16 on device (tolerance 2e-2 >> fp16 rounding 2^-11); host pads
  inputs to fp16 and upcasts outputs to fp32.
"""
import sys

if '/opt/trn_rl_repo' not in sys.path:
    sys.path.insert(0, '/opt/trn_rl_repo')

import numpy as np

G, N1, N2, K = 32, 2048, 2048, 64
N_CORES = 8
GPC = G // N_CORES

_cache = {}


def _build(m1, m2):
    from concourse import bacc
    import concourse.tile as tile
    import concourse.mybir as mybir

    F = mybir.dt.float16
    m2k = m2 * K
    nc = bacc.Bacc("TRN2", target_bir_lowering=False, debug=False)
    p1 = nc.dram_tensor("p1", [3, GPC, m1, K], F, kind="ExternalInput")
    p2f = nc.dram_tensor("p2f", [3, GPC, 1, m2k], F, kind="ExternalInput")
    out = nc.dram_tensor("out", [3, GPC, 2, m1, m2k], F,
                         kind="ExternalOutput")

    with tile.TileContext(nc) as tc:
        with tc.tile_pool(name="in1", bufs=2) as pin1, \
             tc.tile_pool(name="exp0", bufs=6) as pexp0:
            it = 0
            for t in range(3):
                for g in range(GPC):
                    # ch1: one engine-free HBM->HBM row-broadcast DMA
                    nc.gpsimd.dma_start(
                        out[t, g, 1],
                        p2f[t, g].broadcast_to([m1, m2k]))

                    # ch0: load + one-shot broadcast copy + one DMA
                    t1 = pin1.tile([m1, K], F)
                    nc.sync.dma_start(t1[:], p1[t, g])
                    e0 = pexp0.tile([m1, m2k], F)
                    nc.vector.tensor_copy(
                        e0[:], t1[:].unsqueeze(1).broadcast_to([m1, m2, K]))
                    nc.gpsimd.dma_start(out[t, g, 0], e0[:])
                    it += 1
    nc.compile()
    return nc


def _pad_groups_np(x, ids, m):
    counts = np.bincount(ids, minlength=G)
    starts = np.cumsum(counts) - counts
    pos = np.arange(ids.shape[0]) - starts[ids]
    outp = np.zeros((G, m, x.shape[1]), np.float16)
    outp[ids, pos] = x.astype(np.float16)
    return outp


def _make_in_maps(inputs):
    ids1 = np.asarray(inputs['ids1']).astype(np.int64)
    ids2 = np.asarray(inputs['ids2']).astype(np.int64)
    m1 = int(inputs['maxcount1'])
    m2 = int(inputs['maxcount2'])
    xs1 = [np.asarray(inputs[n]) for n in ('x_f_1', 'x_e_1', 'x_v_1')]
    xs2 = [np.asarray(inputs[n]) for n in ('x_f_2', 'x_e_2', 'x_v_2')]
    pad1 = np.stack([_pad_groups_np(x, ids1, m1) for x in xs1])
    pad2f = np.stack([_pad_groups_np(x, ids2, m2) for x in xs2]).reshape(
        3, G, 1, m2 * K)
    in_maps = [
        {"p1": np.ascontiguousarray(pad1[:, c * GPC:(c + 1) * GPC]),
         "p2f": np.ascontiguousarray(pad2f[:, c * GPC:(c + 1) * GPC])}
        for c in range(N_CORES)
    ]
    return in_maps, m1, m2


def kernel(**inputs):
    from concourse.bass_utils import run_bass_kernel_spmd

    in_maps, m1, m2 = _make_in_maps(inputs)
    key = (m1, m2)
    if key not in _cache:
        _cache[key] = _build(m1, m2)
    nc = _cache[key]

    res = run_bass_kernel_spmd(nc, in_maps, core_ids=list(range(N_CORES)))

    full = np.empty((3, G, 2, m1, m2, K), np.float32)
    for c in range(N_CORES):
        full[:, c * GPC:(c + 1) * GPC] = np.asarray(
            res.results[c]["out"]).astype(np.float32).reshape(
            3, GPC, 2, m1, m2, K)
    return full[0], full[1], full[2]
